# revision 6
# baseline (speedup 1.0000x reference)
"""Causal MHA (B=4, H=16, S=2048, D=64) on 8 TRN2 NeuronCores — v2.

Sharding: B*H = 64 head-slices -> 8 per core, processed as 4 head-PAIRS.

Per-core algorithm (per head pair A,B):
  - Cast Q/K f32->bf16 into DRAM scratch [S, 128] = [A | B] (64 cols each),
    one XBAR-transpose -> qt2/kt2 [128, S] with A in partitions 0:64 and B
    in 64:128 (d on partitions).
  - Scores: PE row-tiling packs both heads into one pass: per k-block kb and
    512-col q-window, matmul tile (0,0) contracts partitions 0:64 (head A)
    and tile (64,0) contracts 64:128 (head B); both stream the same q-window
    concurrently, outputs to adjacent PSUM banks of one [128, 2, 512] tile.
  - exp: alternates between ScalarE (exact, activation Exp) and DVE
    (Schraudolph: bf16 bits = int16(x*a + b), one tensor_scalar op) to halve
    the softmax-exp wall time. Output ut[kb] [128, 2, L] bf16.
  - PV: per head, O accumulates in one PSUM tile po [128, 16, 128] (4 banks);
    chain qb uses weights ut[kp] and rhs [V|1]. Normalization is batched per
    bank-group of 4 qb: one reciprocal + one scalar_tensor_tensor with a
    broadcast multiplier.
"""

import numpy as np

import bass_rust as _bass_rust
import concourse.bass as bass
import concourse.tile as tile
from concourse import mybir
from concourse.bass_utils import run_bass_kernel_spmd
from concourse.masks import make_upper_triangular

F32 = mybir.dt.float32
BF16 = mybir.dt.bfloat16
I16 = mybir.dt.int16

B, H, S, D = 4, 16, 2048, 64
N_CORES = 8
HEADS_PER_CORE = B * H // N_CORES  # 8
NB = S // 128  # 16 blocks of 128
SCALE = 1.0 / np.sqrt(np.float32(D))  # 0.125
LOG2E = 1.4426950408889634
A_SCH = float(SCALE * LOG2E * 128.0)  # bf16 exponent domain (2^7 mantissa)
B_SCH = 16252.0  # 127*128 minus minimax shift (~4), +0.5 rounding fudge
WARMUP_MM = 64  # dummy matmuls to hold the PE HAM window open at start


def build_nc(n_heads: int = HEADS_PER_CORE):
    assert n_heads % 2 == 0
    npairs = n_heads // 2
    nc = bass.Bass("TRN2", target_bir_lowering=False)
    q_d = nc.dram_tensor("queries", [n_heads, S, D], F32, kind="ExternalInput")
    k_d = nc.dram_tensor("keys", [n_heads, S, D], F32, kind="ExternalInput")
    v_d = nc.dram_tensor("values", [n_heads, S, D], F32, kind="ExternalInput")
    o_d = nc.dram_tensor("out", [n_heads, S, D], F32, kind="ExternalOutput")

    # [h, p, n, d] views: s = n*128 + p
    v_r = v_d[:].rearrange("h (n p) d -> h p n d", p=128)
    o_r = o_d[:].rearrange("h (n p) d -> h p n d", p=128)

    # greedy softmax-exp engine balancer (ns per engine); DVE starts with a
    # reserve for its reciprocal/normalize work.
    eng_ns = {"S": 0.0, "D": 14000.0}

    def exp_engine(cols):
        cs = 170.0 + cols * 0.833  # ScalarE: fixed overhead + 1 elem/col @1.2G
        cd = 172.0 + cols * 1.042  # DVE: fixed overhead + 1 elem/col @0.96G
        if eng_ns["S"] + cs <= eng_ns["D"] + cd:
            eng_ns["S"] += cs
            return "S"
        eng_ns["D"] += cd
        return "D"

    with tile.TileContext(nc) as tc:
        with (
            tc.tile_pool(name="const", bufs=1) as constp,
            tc.tile_pool(name="scr", bufs=3, space="DRAM") as scrp,
            tc.tile_pool(name="tp", bufs=2) as tpp,
            tc.tile_pool(name="vpool", bufs=2) as vpp,
            tc.tile_pool(name="ut", bufs=2) as utp,
            tc.tile_pool(name="oh", bufs=2) as ohp,
            tc.tile_pool(name="rz", bufs=2) as rzp,
            tc.tile_pool(name="ps_s", bufs=2, space="PSUM") as ps_s,
            tc.tile_pool(name="ps_o", bufs=1, space="PSUM") as ps_o,
        ):
            trimask = constp.tile([128, 128], BF16)
            make_upper_triangular(nc, trimask, val=1.0, diag=True)

            # Keep the PE busy (HAM warm) while the first pair's Q/K DMA
            # pipeline fills; garbage matmuls into a scores PSUM slot.
            wps = ps_s.tile([128, 2, 512], F32, tag="sAB")
            for _ in range(WARMUP_MM):
                nc.tensor.matmul(
                    wps[:, 0, 0:128], lhsT=trimask, rhs=trimask,
                    start=True, stop=True,
                )

            scrs = {}
            tps = {}
            vps = {}

            def issue_casts(p):
                # [S, 128] = [head A | head B] bf16 scratch for Q and K.
                scq = scrp.tile([S, 128], BF16, tag="scq")
                sck = scrp.tile([S, 128], BF16, tag="sck")
                nc.gpsimd.dma_start(out=scq[:, 0:64], in_=q_d[2 * p])
                nc.gpsimd.dma_start(out=scq[:, 64:128], in_=q_d[2 * p + 1])
                nc.gpsimd.dma_start(out=sck[:, 0:64], in_=k_d[2 * p])
                nc.gpsimd.dma_start(out=sck[:, 64:128], in_=k_d[2 * p + 1])
                scrs[p] = (scq, sck)

            def issue_xpose(p):
                scq, sck = scrs.pop(p)
                qt2 = tpp.tile([128, S], BF16, tag="qt")
                kt2 = tpp.tile([128, S], BF16, tag="kt")
                nc.sync.dma_start(out=qt2, in_=scq, transpose=True)
                nc.sync.dma_start(out=kt2, in_=sck, transpose=True)
                tps[p] = (qt2, kt2)

            def issue_v(p):
                vpA = vpp.tile([128, NB, D + 1], BF16, tag="vA")
                vpB = vpp.tile([128, NB, D + 1], BF16, tag="vB")
                nc.gpsimd.dma_start(out=vpA[:, :, 0:D], in_=v_r[2 * p])
                nc.gpsimd.dma_start(out=vpB[:, :, 0:D], in_=v_r[2 * p + 1])
                nc.gpsimd.memset(vpA[:, :, D : D + 1], 1.0)
                nc.gpsimd.memset(vpB[:, :, D : D + 1], 1.0)
                vps[p] = (vpA, vpB)

            for p in range(min(2, npairs)):
                issue_casts(p)
            for p in range(min(1, npairs)):
                issue_xpose(p)
                issue_v(p)

            def pv_chain(po, uts, vp, hsel, qb):
                for kp in range(qb + 1):
                    c = (qb - kp) * 128
                    nc.tensor.matmul(
                        po[:, qb, 0:65],
                        lhsT=uts[kp][:, hsel, c : c + 128],
                        rhs=vp[:, kp, :],
                        start=(kp == 0),
                        stop=(kp == qb),
                    )

            def norm_group(po, rz, oh, g):
                g0, g1 = 4 * g, 4 * g + 4
                nc.vector.reciprocal(rz[:, g0:g1, :], po[:, g0:g1, 64:65])
                nc.vector.scalar_tensor_tensor(
                    out=oh[:, g0:g1, :],
                    in0=po[:, g0:g1, 0:64],
                    scalar=1.0,
                    in1=rz[:, g0:g1, :].broadcast_to((128, 4, 64)),
                    op0=mybir.AluOpType.mult,
                    op1=mybir.AluOpType.mult,
                )

            for p in range(npairs):
                if p + 2 < npairs:
                    issue_casts(p + 2)
                if p + 1 < npairs:
                    issue_xpose(p + 1)
                    issue_v(p + 1)
                qt2, kt2 = tps.pop(p)
                vpA, vpB = vps.pop(p)

                poA = ps_o.tile([128, NB, 128], F32, tag="po")
                rzA = rzp.tile([128, NB, 1], F32, tag="rzA")
                ohA = ohp.tile([128, NB, D], F32, tag="ohA")
                uts = []
                for kb in range(NB):
                    L = S - kb * 128
                    kbs = slice(kb * 128, (kb + 1) * 128)
                    ut = utp.tile([128, 2, L], BF16, tag=f"ut{kb}")
                    uts.append(ut)
                    off = 0
                    while off < L:
                        w = min(512, L - off)
                        ps = ps_s.tile([128, 2, 512], F32, tag="sAB")
                        q0 = kb * 128 + off
                        nc.tensor.matmul(
                            ps[:, 0, 0:w],
                            lhsT=kt2[0:64, kbs],
                            rhs=qt2[0:64, q0 : q0 + w],
                            start=True,
                            stop=True,
                        )
                        nc.tensor.matmul(
                            ps[:, 1, 0:w],
                            lhsT=kt2[64:128, kbs],
                            rhs=qt2[64:128, q0 : q0 + w],
                            start=True,
                            stop=True,
                        )
                        if exp_engine(2 * w) == "S":
                            nc.scalar.activation(
                                out=ut[:, :, off : off + w],
                                in_=ps[:, :, 0:w],
                                func=mybir.ActivationFunctionType.Exp,
                                scale=float(SCALE),
                            )
                        else:
                            nc.vector.tensor_scalar(
                                ut[:, :, off : off + w].bitcast(I16),
                                ps[:, :, 0:w],
                                A_SCH,
                                B_SCH,
                                mybir.AluOpType.mult,
                                mybir.AluOpType.add,
                            )
                        off += w
                    # mask diagonal block: keep k <= q (partition <= free)
                    nc.gpsimd.tensor_mul(ut[:, 0, 0:128], ut[:, 0, 0:128], trimask)
                    nc.gpsimd.tensor_mul(ut[:, 1, 0:128], ut[:, 1, 0:128], trimask)
                    # head A's PV chain for qb=kb is now unblocked
                    pv_chain(poA, uts, vpA, 0, kb)
                    if kb % 4 == 3:
                        norm_group(poA, rzA, ohA, kb // 4)
                nc.sync.dma_start(out=o_r[2 * p], in_=ohA)

                poB = ps_o.tile([128, NB, 128], F32, tag="po")
                rzB = rzp.tile([128, NB, 1], F32, tag="rzB")
                ohB = ohp.tile([128, NB, D], F32, tag="ohB")
                for qb in range(NB):
                    pv_chain(poB, uts, vpB, 1, qb)
                    if qb % 4 == 3:
                        norm_group(poB, rzB, ohB, qb // 4)
                nc.sync.dma_start(out=o_r[2 * p + 1], in_=ohB)
    # This walrus build allows at most one sem wait per instruction, and
    # moving waits onto plain NOPs is unsound on the PE (its queue pulls
    # LDWEIGHTS ahead of wait-carrying NOPs -> stale weight reads). Use the
    # toolchain's own passes: waits move onto the ldweights instructions
    # themselves, and remaining multi-waits split via event semaphores.
    _bass_rust.move_matmul_waits_to_ldweights(nc.m)
    _bass_rust.generate_event_semaphores(nc)
    return nc


_NC_CACHE = {}


def _get_nc(n_heads: int = HEADS_PER_CORE):
    if n_heads not in _NC_CACHE:
        _NC_CACHE[n_heads] = build_nc(n_heads)
    return _NC_CACHE[n_heads]


def make_in_maps(queries, keys, values):
    qf = np.ascontiguousarray(
        np.asarray(queries, dtype=np.float32).reshape(B * H, S, D)
    )
    kf = np.ascontiguousarray(np.asarray(keys, dtype=np.float32).reshape(B * H, S, D))
    vf = np.ascontiguousarray(
        np.asarray(values, dtype=np.float32).reshape(B * H, S, D)
    )
    n = HEADS_PER_CORE
    return [
        {
            "queries": qf[i * n : (i + 1) * n],
            "keys": kf[i * n : (i + 1) * n],
            "values": vf[i * n : (i + 1) * n],
        }
        for i in range(N_CORES)
    ]


def kernel(keys, queries, values, head_dim=None, **_ignored):
    nc = _get_nc()
    in_maps = make_in_maps(queries, keys, values)
    res = run_bass_kernel_spmd(nc, in_maps, core_ids=list(range(N_CORES)))
    out = np.concatenate([res.results[i]["out"] for i in range(N_CORES)], axis=0)
    return out.reshape(B, H, S, D).astype(np.float32)


# revision 7
# speedup vs baseline: 1.0884x; 1.0884x over previous
"""Causal MHA (B=4, H=16, S=2048, D=64) on 8 TRN2 NeuronCores — v2.

Sharding: B*H = 64 head-slices -> 8 per core, processed as 4 head-PAIRS.

Per-core algorithm (per head pair A,B):
  - Cast Q/K f32->bf16 into DRAM scratch [S, 128] = [A | B] (64 cols each),
    one XBAR-transpose -> qt2/kt2 [128, S] with A in partitions 0:64 and B
    in 64:128 (d on partitions).
  - Scores: PE row-tiling packs both heads into one pass: per k-block kb and
    512-col q-window, matmul tile (0,0) contracts partitions 0:64 (head A)
    and tile (64,0) contracts 64:128 (head B); both stream the same q-window
    concurrently, outputs to adjacent PSUM banks of one [128, 2, 512] tile.
  - exp: alternates between ScalarE (exact, activation Exp) and DVE
    (Schraudolph: bf16 bits = int16(x*a + b), one tensor_scalar op) to halve
    the softmax-exp wall time. Output ut[kb] [128, 2, L] bf16.
  - PV: per head, O accumulates in one PSUM tile po [128, 16, 128] (4 banks);
    chain qb uses weights ut[kp] and rhs [V|1]. Normalization is batched per
    bank-group of 4 qb: one reciprocal + one scalar_tensor_tensor with a
    broadcast multiplier.
"""

import numpy as np

import bass_rust as _bass_rust
import concourse.bass as bass
import concourse.tile as tile
from concourse import mybir
from concourse.bass_utils import run_bass_kernel_spmd
from concourse.masks import make_upper_triangular

F32 = mybir.dt.float32
BF16 = mybir.dt.bfloat16
I16 = mybir.dt.int16

B, H, S, D = 4, 16, 2048, 64
N_CORES = 8
HEADS_PER_CORE = B * H // N_CORES  # 8
NB = S // 128  # 16 blocks of 128
SCALE = 1.0 / np.sqrt(np.float32(D))  # 0.125
LOG2E = 1.4426950408889634
A_SCH = float(SCALE * LOG2E * 128.0)  # bf16 exponent domain (2^7 mantissa)
B_SCH = 16252.0  # 127*128 minus minimax shift (~4), +0.5 rounding fudge
WARMUP_MM = 64  # dummy matmuls to hold the PE HAM window open at start


def build_nc(n_heads: int = HEADS_PER_CORE):
    assert n_heads % 2 == 0
    npairs = n_heads // 2
    nc = bass.Bass("TRN2", target_bir_lowering=False)
    q_d = nc.dram_tensor("queries", [n_heads, S, D], F32, kind="ExternalInput")
    k_d = nc.dram_tensor("keys", [n_heads, S, D], F32, kind="ExternalInput")
    v_d = nc.dram_tensor("values", [n_heads, S, D], F32, kind="ExternalInput")
    o_d = nc.dram_tensor("out", [n_heads, S, D], F32, kind="ExternalOutput")

    # [h, p, n, d] views: s = n*128 + p
    v_r = v_d[:].rearrange("h (n p) d -> h p n d", p=128)
    o_r = o_d[:].rearrange("h (n p) d -> h p n d", p=128)

    # greedy softmax-exp engine balancer (ns per engine); DVE starts with a
    # reserve for its reciprocal/normalize work.
    eng_ns = {"S": 0.0, "D": 3000.0}

    def exp_engine(cols):
        cs = 170.0 + cols * 0.833  # ScalarE: fixed overhead + 1 elem/col @1.2G
        cd = 172.0 + cols * 1.042  # DVE: fixed overhead + 1 elem/col @0.96G
        if eng_ns["S"] + cs <= eng_ns["D"] + cd:
            eng_ns["S"] += cs
            return "S"
        eng_ns["D"] += cd
        return "D"

    with tile.TileContext(nc) as tc:
        with (
            tc.tile_pool(name="const", bufs=1) as constp,
            tc.tile_pool(name="scr", bufs=3, space="DRAM") as scrp,
            tc.tile_pool(name="tp", bufs=2) as tpp,
            tc.tile_pool(name="vpool", bufs=2) as vpp,
            tc.tile_pool(name="ut", bufs=2) as utp,
            tc.tile_pool(name="oh", bufs=2) as ohp,
            tc.tile_pool(name="rz", bufs=2) as rzp,
            tc.tile_pool(name="ps_s", bufs=3, space="PSUM") as ps_s,
            tc.tile_pool(name="ps_o", bufs=1, space="PSUM") as ps_o,
        ):
            trimask = constp.tile([128, 128], BF16)
            make_upper_triangular(nc, trimask, val=1.0, diag=True)

            # Keep the PE busy (HAM warm) while the first pair's Q/K DMA
            # pipeline fills; garbage matmuls into a scores PSUM slot.
            wps = ps_s.tile([128, 2, 512], F32, tag="sAB")
            for _ in range(WARMUP_MM):
                nc.tensor.matmul(
                    wps[:, 0, 0:128], lhsT=trimask, rhs=trimask,
                    start=True, stop=True,
                )

            scrs = {}
            tps = {}
            vps = {}

            def issue_casts(p, nsplit=1):
                # [S, 128] = [head A | head B] bf16 scratch for Q and K.
                # SP HWDGE triggers are cheap; keeping loads off gpsimd's
                # SWDGE leaves its FIFO free for the trimask muls. Pair 0 is
                # split into row quarters so the first transpose (and the
                # first score matmul) can start ~4x earlier.
                scq = scrp.tile([S, 128], BF16, tag="scq")
                sck = scrp.tile([S, 128], BF16, tag="sck")
                r = S // nsplit
                for i in range(nsplit):
                    sl = slice(i * r, (i + 1) * r)
                    nc.gpsimd.dma_start(out=scq[sl, 0:64], in_=q_d[2 * p, sl])
                    nc.gpsimd.dma_start(out=scq[sl, 64:128], in_=q_d[2 * p + 1, sl])
                    nc.gpsimd.dma_start(out=sck[sl, 0:64], in_=k_d[2 * p, sl])
                    nc.gpsimd.dma_start(out=sck[sl, 64:128], in_=k_d[2 * p + 1, sl])
                scrs[p] = (scq, sck, nsplit)

            def issue_xpose(p):
                scq, sck, nsplit = scrs.pop(p)
                qt2 = tpp.tile([128, S], BF16, tag="qt")
                kt2 = tpp.tile([128, S], BF16, tag="kt")
                r = S // nsplit
                for i in range(nsplit):
                    sl = slice(i * r, (i + 1) * r)
                    nc.sync.dma_start(out=kt2[:, sl], in_=sck[sl, :], transpose=True)
                    nc.sync.dma_start(out=qt2[:, sl], in_=scq[sl, :], transpose=True)
                tps[p] = (qt2, kt2)

            def issue_v(p):
                vpA = vpp.tile([128, NB, D + 1], BF16, tag="vA")
                vpB = vpp.tile([128, NB, D + 1], BF16, tag="vB")
                nc.gpsimd.dma_start(out=vpA[:, :, 0:D], in_=v_r[2 * p])
                nc.gpsimd.dma_start(out=vpB[:, :, 0:D], in_=v_r[2 * p + 1])
                nc.gpsimd.memset(vpA[:, :, D : D + 1], 1.0)
                nc.gpsimd.memset(vpB[:, :, D : D + 1], 1.0)
                vps[p] = (vpA, vpB)

            issue_casts(0, nsplit=4)
            issue_xpose(0)
            issue_v(0)
            if npairs > 1:
                issue_casts(1)

            def pv_chain(po, uts, vp, hsel, qb):
                for kp in range(qb + 1):
                    c = (qb - kp) * 128
                    nc.tensor.matmul(
                        po[:, qb % 8, 0:65],
                        lhsT=uts[kp][:, hsel, c : c + 128],
                        rhs=vp[:, kp, :],
                        start=(kp == 0),
                        stop=(kp == qb),
                    )

            def norm_group(po, rz, oh, g):
                g0, g1 = 4 * g, 4 * g + 4
                p0, p1 = (4 * g) % 8, (4 * g) % 8 + 4
                nc.vector.reciprocal(rz[:, g0:g1, :], po[:, p0:p1, 64:65])
                nc.vector.scalar_tensor_tensor(
                    out=oh[:, g0:g1, :],
                    in0=po[:, p0:p1, 0:64],
                    scalar=1.0,
                    in1=rz[:, g0:g1, :].broadcast_to((128, 4, 64)),
                    op0=mybir.AluOpType.mult,
                    op1=mybir.AluOpType.mult,
                )

            for p in range(npairs):
                if p + 1 < npairs:
                    issue_xpose(p + 1)
                qt2, kt2 = tps.pop(p)
                vpA, vpB = vps.pop(p)

                poA = ps_o.tile([128, 8, 128], F32, tag="po")
                rzA = rzp.tile([128, NB, 1], F32, tag="rzA")
                ohA = ohp.tile([128, NB, D], F32, tag="ohA")
                uts = []
                for kb in range(NB):
                    if kb == 8:
                        poA = ps_o.tile([128, 8, 128], F32, tag="po")
                    L = S - kb * 128
                    kbs = slice(kb * 128, (kb + 1) * 128)
                    ut = utp.tile([128, 2, L], BF16, tag=f"ut{kb}")
                    uts.append(ut)
                    off = 0
                    while off < L:
                        w = min(512, L - off)
                        ps = ps_s.tile([128, 2, 512], F32, tag="sAB")
                        q0 = kb * 128 + off
                        nc.tensor.matmul(
                            ps[:, 0, 0:w],
                            lhsT=kt2[0:64, kbs],
                            rhs=qt2[0:64, q0 : q0 + w],
                            start=True,
                            stop=True,
                        )
                        nc.tensor.matmul(
                            ps[:, 1, 0:w],
                            lhsT=kt2[64:128, kbs],
                            rhs=qt2[64:128, q0 : q0 + w],
                            start=True,
                            stop=True,
                        )
                        if exp_engine(2 * w) == "S":
                            nc.scalar.activation(
                                out=ut[:, :, off : off + w],
                                in_=ps[:, :, 0:w],
                                func=mybir.ActivationFunctionType.Exp,
                                scale=float(SCALE),
                            )
                        else:
                            nc.vector.tensor_scalar(
                                ut[:, :, off : off + w].bitcast(I16),
                                ps[:, :, 0:w],
                                A_SCH,
                                B_SCH,
                                mybir.AluOpType.mult,
                                mybir.AluOpType.add,
                            )
                        off += w
                    # mask diagonal block: keep k <= q (partition <= free)
                    nc.gpsimd.tensor_mul(ut[:, 0, 0:128], ut[:, 0, 0:128], trimask)
                    nc.gpsimd.tensor_mul(ut[:, 1, 0:128], ut[:, 1, 0:128], trimask)
                    # head A's PV chain for qb=kb is now unblocked
                    pv_chain(poA, uts, vpA, 0, kb)
                    if kb % 4 == 3:
                        norm_group(poA, rzA, ohA, kb // 4)
                nc.sync.dma_start(out=o_r[2 * p], in_=ohA)
                # heavy SWDGE descriptor builds go behind this pair's
                # trimask muls in the gpsimd FIFO
                if p + 1 < npairs:
                    issue_v(p + 1)
                if p + 2 < npairs:
                    issue_casts(p + 2)

                poB = ps_o.tile([128, 8, 128], F32, tag="po")
                rzB = rzp.tile([128, NB, 1], F32, tag="rzB")
                ohB = ohp.tile([128, NB, D], F32, tag="ohB")
                for qb in range(NB):
                    if qb == 8:
                        poB = ps_o.tile([128, 8, 128], F32, tag="po")
                    pv_chain(poB, uts, vpB, 1, qb)
                    if qb % 4 == 3:
                        norm_group(poB, rzB, ohB, qb // 4)
                nc.sync.dma_start(out=o_r[2 * p + 1], in_=ohB)
    # This walrus build allows at most one sem wait per instruction, and
    # moving waits onto plain NOPs is unsound on the PE (its queue pulls
    # LDWEIGHTS ahead of wait-carrying NOPs -> stale weight reads). Use the
    # toolchain's own passes: waits move onto the ldweights instructions
    # themselves, and remaining multi-waits split via event semaphores.
    _bass_rust.move_matmul_waits_to_ldweights(nc.m)
    _bass_rust.generate_event_semaphores(nc)
    return nc


_NC_CACHE = {}


def _get_nc(n_heads: int = HEADS_PER_CORE):
    if n_heads not in _NC_CACHE:
        _NC_CACHE[n_heads] = build_nc(n_heads)
    return _NC_CACHE[n_heads]


def make_in_maps(queries, keys, values):
    qf = np.ascontiguousarray(
        np.asarray(queries, dtype=np.float32).reshape(B * H, S, D)
    )
    kf = np.ascontiguousarray(np.asarray(keys, dtype=np.float32).reshape(B * H, S, D))
    vf = np.ascontiguousarray(
        np.asarray(values, dtype=np.float32).reshape(B * H, S, D)
    )
    n = HEADS_PER_CORE
    return [
        {
            "queries": qf[i * n : (i + 1) * n],
            "keys": kf[i * n : (i + 1) * n],
            "values": vf[i * n : (i + 1) * n],
        }
        for i in range(N_CORES)
    ]


def kernel(keys, queries, values, head_dim=None, **_ignored):
    nc = _get_nc()
    in_maps = make_in_maps(queries, keys, values)
    res = run_bass_kernel_spmd(nc, in_maps, core_ids=list(range(N_CORES)))
    out = np.concatenate([res.results[i]["out"] for i in range(N_CORES)], axis=0)
    return out.reshape(B, H, S, D).astype(np.float32)
